# revision 5
# baseline (speedup 1.0000x reference)
"""CoFix3D decoder head (conv3x3 -> ReLU -> conv3x3 -> sigmoid -> 3x3 NMS ->
top-200 proposals + embedding MLP) on 8 Trainium2 NeuronCores.

Sharding: data-parallel over (batch, half-image).  Each core runs a fused
Bass/Tile kernel over half of one sample (180 output rows + conv halo):

  * conv1 (128->128, 3x3, fp32): 9 PSUM-accumulated matmuls per row.
  * conv2 (128->10, 3x3, fp32): the three dx-taps are stacked into one
    M=74 stationary operand (blocks at partitions 0/32/64), so only 3
    matmuls per row; the dx-shifted partials are combined with 2 DVE adds.
  * sigmoid (ACT) rows are DMA-scattered into a [120-partition] layout
    (partition = block*10+class, 15 rows + halo slots per partition) so the
    3x3 local-max NMS runs at full 120-lane DVE width.
  * NMS mask + masked heatmap, plus top-8-per-(partition x 1360-col chunk)
    candidate extraction (DVE max8/max_index).

All matmuls are fp32 (PE 2-pass mode): the NMS mask bits and the top-200
ranking are only reproducible at fp32 accuracy (top-200 adjacent score gaps
go down to ~6e-7; tf32-style fp32r flips thousands of mask bits).

The host merges the per-strip candidates into the exact per-sample top-200
(with a sufficiency guard + exact fallback from the returned heatmap) and
runs the tiny proposal-embedding MLP (200x256 GEMM per sample) in numpy.
"""
import os
import sys
import math

sys.path.insert(0, "/opt/trn_rl_repo")

import numpy as np
import concourse.bacc as bacc
import concourse.tile as tile
from concourse import mybir

f32 = mybir.dt.float32
u32 = mybir.dt.uint32

H = 360
W = 360
C = 128
NC_ = 10
NPROP = 200
NBLK = 12
WP = 362  # padded row width
OFFS = [(dy, dx) for dy in (-1, 0, 1) for dx in (-1, 0, 1)]
NEG = -1.0e30

_CACHE = {}


# --------------------------------------------------------------------------
# device module
# --------------------------------------------------------------------------

def build_strip_module(nblk=NBLK, include_topk=True):
    """Strip module. Output rows = 15*nblk; feat strip rows = out+6."""
    OUT = 15 * nblk
    FR = OUT + 6           # feat strip rows
    NPART = 10 * nblk      # hm partitions (blk*10 + cls)
    HMW = 17 * WP          # hm cols per partition (15 owned + 2 halo slots)
    TKW = 5440             # candidate-buffer cols: 15*362=5430 data + 10 pad
    NFT = (FR + 3) // 4    # feat tiles of 4 rows

    nc = bacc.Bacc("TRN2")
    feat_d = nc.dram_tensor("feat", [128, FR, W], f32, kind="ExternalInput")
    w1_d = nc.dram_tensor("w1s", [128, 9 * 128], f32, kind="ExternalInput")
    w2_d = nc.dram_tensor("w2s", [128, 3 * 74], f32, kind="ExternalInput")
    b1_d = nc.dram_tensor("b1", [128, 1], f32, kind="ExternalInput")
    b2_d = nc.dram_tensor("b2", [10, 1], f32, kind="ExternalInput")
    edge_d = nc.dram_tensor("edge", [NPART, 2], f32, kind="ExternalInput")
    c1edge_d = nc.dram_tensor("c1edge", [128, 2], f32, kind="ExternalInput")
    hm_out_d = nc.dram_tensor("hm_out", [10, OUT, W], f32, kind="ExternalOutput")
    if include_topk:
        cvals_d = nc.dram_tensor("cand_vals", [128, 32], f32, kind="ExternalOutput")
        cidx_d = nc.dram_tensor("cand_idx", [128, 32], u32, kind="ExternalOutput")

    def dests(y):
        """hm (blk, slot) destinations for conv2 strip row y."""
        d = []
        q = y - 3
        if 0 <= q < OUT:
            d.append((q // 15, q % 15 + 1))
        if (q + 1) % 15 == 0 and 0 <= (q + 1) // 15 <= nblk - 1:
            d.append(((q + 1) // 15, 0))
        if q >= 0 and q % 15 == 0 and q // 15 - 1 >= 0:
            d.append((q // 15 - 1, 16))
        return d

    with tile.TileContext(nc) as tc:
        with tc.tile_pool(name="consts", bufs=1) as consts, \
             tc.tile_pool(name="featp", bufs=6) as featp, \
             tc.tile_pool(name="c1p", bufs=10) as c1p, \
             tc.tile_pool(name="hrowp", bufs=6) as hrowp, \
             tc.tile_pool(name="dtp", bufs=6) as dtp, \
             tc.tile_pool(name="ps1", bufs=4, space="PSUM") as ps1, \
             tc.tile_pool(name="ps2", bufs=4, space="PSUM") as ps2:

            w1t = consts.tile([128, 9 * 128], f32, tag="w1t")
            w2t = consts.tile([128, 3 * 74], f32, tag="w2t")
            b1t = consts.tile([128, 1], f32, tag="b1t")
            b2t = consts.tile([10, 1], f32, tag="b2t")
            edget = consts.tile([NPART, 2], f32, tag="edget")
            c1edget = consts.tile([128, 2], f32, tag="c1edget")
            hm = consts.tile([NPART, HMW], f32, tag="hm")
            topkin = consts.tile([128, TKW], f32, tag="topkin")
            vbuf = consts.tile([NPART, 15 * WP], f32, tag="vbuf")
            pbuf = consts.tile([NPART, 15 * W], f32, tag="pbuf")

            nc.sync.dma_start(w1t, w1_d[:, :])
            nc.sync.dma_start(w2t, w2_d[:, :])
            nc.sync.dma_start(b1t, b1_d[:, :])
            nc.sync.dma_start(b2t, b2_d[:, :])
            nc.sync.dma_start(edget, edge_d[:, :])
            nc.sync.dma_start(c1edget, c1edge_d[:, :])

            hm3 = hm.rearrange("p (s x) -> p s x", x=WP)
            nc.vector.memset(hm3[:, :, 0:1], NEG)
            nc.vector.memset(hm3[:, :, 361:362], NEG)
            nc.vector.memset(topkin, 0.0)

            feat_tiles = {}
            next_ft = [0]

            def load_feat_upto(t_hi):
                while next_ft[0] <= min(t_hi, NFT - 1):
                    t = next_ft[0]
                    ft = featp.tile([128, 4 * WP], f32, tag="ft")
                    r0, r1 = 4 * t, min(4 * t + 4, FR)
                    ft3 = ft.rearrange("p (r x) -> p r x", x=WP)
                    nc.sync.dma_start(ft3[:, 0:r1 - r0, 1:361], feat_d[:, r0:r1, :])
                    # zero the 2 pad columns of each row slot
                    nc.vector.memset(ft3[:, :, 0:1], 0.0)
                    nc.vector.memset(ft3[:, :, 361:362], 0.0)
                    feat_tiles[t] = ft
                    next_ft[0] += 1

            c1_tiles = {}

            for j in range(1, OUT + 5):
                load_feat_upto((j + 1) // 4)
                p1 = ps1.tile([128, W], f32, tag="p1")
                for k, (dy, dx) in enumerate(OFFS):
                    sr = j + dy
                    ft = feat_tiles[sr // 4]
                    s = (sr % 4) * WP + dx + 1
                    nc.tensor.matmul(p1, lhsT=w1t[:, 128 * k:128 * (k + 1)],
                                     rhs=ft[:, s:s + W],
                                     start=(k == 0), stop=(k == 8))
                c1 = c1p.tile([128, WP], f32, tag="c1")
                nc.vector.memset(c1[:, 0:1], 0.0)
                nc.vector.memset(c1[:, 361:362], 0.0)
                nc.scalar.activation(c1[:, 1:361], p1,
                                     mybir.ActivationFunctionType.Relu,
                                     bias=b1t[:, 0:1], scale=1.0)
                # conv1 rows outside the global image must be ZERO for conv2
                # (the reference zero-pads each conv independently)
                if j in (1, 2):
                    nc.vector.tensor_mul(c1[:, 1:361], c1[:, 1:361],
                                         c1edget[:, 0:1].to_broadcast([128, 360]))
                elif j in (OUT + 3, OUT + 4):
                    nc.vector.tensor_mul(c1[:, 1:361], c1[:, 1:361],
                                         c1edget[:, 1:2].to_broadcast([128, 360]))
                c1_tiles[j] = c1
                y = j - 1
                if y < 2 or y >= OUT + 4:
                    continue
                # conv2 with the 3 dx-taps stacked into one M=74 lhsT
                # (blocks at partitions 0/32/64); dy accumulates in PSUM.
                p2 = ps2.tile([128, 512], f32, tag="p2")
                for dyi, dy in enumerate((-1, 0, 1)):
                    nc.tensor.matmul(p2[0:74, 0:WP],
                                     lhsT=w2t[:, 74 * dyi:74 * (dyi + 1)],
                                     rhs=c1_tiles[y + dy][:, 0:WP],
                                     start=(dyi == 0), stop=(dyi == 2))
                # dense[c, x] = p2[c, x] + p2[32+c, x+1] + p2[64+c, x+2]
                # (DVE can read only ONE input from PSUM -> stage one block
                # through SBUF with an ACT copy)
                dtmp = dtp.tile([10, W], f32, tag="dtmp")
                nc.scalar.copy(dtmp, p2[32:42, 1:W + 1])
                nc.vector.tensor_tensor(dtmp, dtmp, p2[0:10, 0:W],
                                        op=mybir.AluOpType.add)
                nc.vector.tensor_tensor(dtmp, dtmp, p2[64:74, 2:W + 2],
                                        op=mybir.AluOpType.add)
                hrow = hrowp.tile([10, W], f32, tag="hrow")
                nc.scalar.activation(hrow, dtmp,
                                     mybir.ActivationFunctionType.Sigmoid,
                                     bias=b2t[:, 0:1], scale=1.0)
                # engine writes must start 32-aligned; DMA scatters the row
                # into the (blk*10+cls)-partition NMS layout instead.
                for (blk, slot) in dests(y):
                    nc.sync.dma_start(
                        hm[blk * 10:(blk + 1) * 10,
                           slot * WP + 1:slot * WP + 361], hrow)

            # ---- edge invalidation (adds 0 or -2e30 per partition) ----
            nc.vector.tensor_add(hm[:, 0:WP], hm[:, 0:WP],
                                 edget[:, 0:1].to_broadcast([NPART, WP]))
            nc.vector.tensor_add(hm[:, 16 * WP:17 * WP], hm[:, 16 * WP:17 * WP],
                                 edget[:, 1:2].to_broadcast([NPART, WP]))

            # ---- NMS: 3x3 max pool ----
            n15 = 15 * WP
            nc.vector.tensor_tensor(vbuf, hm[:, 0:n15], hm[:, WP:WP + n15],
                                    op=mybir.AluOpType.max)
            nc.vector.tensor_tensor(vbuf, vbuf, hm[:, 2 * WP:2 * WP + n15],
                                    op=mybir.AluOpType.max)
            v3 = vbuf.rearrange("p (r x) -> p r x", x=WP)
            pb3 = pbuf.rearrange("p (r x) -> p r x", x=W)
            nc.vector.tensor_tensor(pb3, v3[:, :, 0:360], v3[:, :, 1:361],
                                    op=mybir.AluOpType.max)
            nc.vector.tensor_tensor(pb3, pb3, v3[:, :, 2:362],
                                    op=mybir.AluOpType.max)
            own = hm3[:, 1:16, 1:361]
            # eq mask computed in place over pbuf (identical in/out APs)
            nc.vector.tensor_tensor(pb3, pb3, own, op=mybir.AluOpType.is_equal)

            # ---- masked scores into candidate-extraction layout ----
            tk3 = topkin[0:NPART, 0:15 * WP].rearrange("p (r x) -> p r x", x=WP)
            nc.vector.tensor_tensor(tk3[:, :, 1:361], pb3, own,
                                    op=mybir.AluOpType.mult)

            # ---- masked heatmap out (per block: [10cls, 15rows, 360]) ----
            for blk in range(nblk):
                nc.sync.dma_start(hm_out_d[:, blk * 15:(blk + 1) * 15, :],
                                  tk3[blk * 10:(blk + 1) * 10, :, 1:361])

            # ---- candidate extraction: top-8 per partition per 1360-col
            # chunk via DVE max8/max_index (4 chunks cover TKW=5440) ----
            if include_topk:
                cvals = consts.tile([128, 32], f32, tag="cvals")
                cidx = consts.tile([128, 32], u32, tag="cidx")
                for ch in range(4):
                    seg = topkin[:, 1360 * ch:1360 * (ch + 1)]
                    nc.vector.max(out=cvals[:, 8 * ch:8 * (ch + 1)], in_=seg)
                    nc.vector.max_index(out=cidx[:, 8 * ch:8 * (ch + 1)],
                                        in_max=cvals[:, 8 * ch:8 * (ch + 1)],
                                        in_values=seg)
                nc.sync.dma_start(cvals_d[:, :], cvals)
                nc.sync.dma_start(cidx_d[:, :], cidx)

    nc.finalize()
    return nc


# --------------------------------------------------------------------------
# host-side sharding helpers
# --------------------------------------------------------------------------

def host_prep_consts(w1, b1, w2, b2):
    w1s = np.ascontiguousarray(
        w1.transpose(1, 2, 3, 0).reshape(128, 9 * 128)).astype(np.float32)
    # stacked-dx conv2 weights: per dy a [128, 74] block with the three
    # dx-taps at column offsets 0/32/64
    w2s = np.zeros((128, 3, 74), np.float32)
    for dyi in range(3):
        for dxi in range(3):
            w2s[:, dyi, 32 * dxi:32 * dxi + 10] = w2[:, :, dyi, dxi].T
    w2s = np.ascontiguousarray(w2s.reshape(128, 3 * 74))
    return {
        "w1s": w1s, "w2s": w2s,
        "b1": b1.reshape(128, 1).astype(np.float32),
        "b2": b2.reshape(10, 1).astype(np.float32),
    }


def host_prep_strip(feat_b, r0, nblk=NBLK):
    """feat_b [128, H, W] -> zero-padded strip [128, 15*nblk+6, W]."""
    OUT = 15 * nblk
    FR = OUT + 6
    hh = feat_b.shape[1]
    fs = np.zeros((128, FR, feat_b.shape[2]), np.float32)
    lo, hi = r0 - 3, r0 + OUT + 3
    s0, s1 = max(lo, 0), min(hi, hh)
    fs[:, s0 - lo:s1 - lo, :] = feat_b[:, s0:s1, :]
    return fs


def host_edge(nblk, top_invalid, bottom_invalid):
    NPART = 10 * nblk
    e = np.zeros((NPART, 2), np.float32)
    if top_invalid:
        e[0:10, 0] = -2.0e30
    if bottom_invalid:
        e[NPART - 10:NPART, 1] = -2.0e30
    return e


def host_c1edge(top_invalid, bottom_invalid):
    e = np.ones((128, 2), np.float32)
    if top_invalid:
        e[:, 0] = 0.0
    if bottom_invalid:
        e[:, 1] = 0.0
    return e


def decode_candidates(cvals, cidx, nblk=NBLK):
    """cvals [128,32] f32 + cidx [128,32] u32 (4 chunks x top8 per partition)
    -> (scores, cls, y_strip, x, valid) flat arrays of 4096 entries."""
    CH = 1360
    part = np.repeat(np.arange(128), 32)
    chunk = np.tile(np.repeat(np.arange(4), 8), 128)
    vals = cvals.reshape(-1)
    col = (cidx.reshape(-1).astype(np.int64) + chunk * CH)
    blk = part // 10
    cls = part % 10
    r15 = col // WP
    xp = col % WP
    valid = (vals > 0) & (xp >= 1) & (xp <= 360) & (r15 < 15) & (part < 10 * nblk)
    return vals, cls, blk * 15 + r15, xp - 1, valid


def _make_in_maps(feat, w1, b1, w2, b2):
    consts = host_prep_consts(w1, b1, w2, b2)
    in_maps = []
    for c in range(8):
        b, half = divmod(c, 2)
        m = dict(consts)
        m["feat"] = host_prep_strip(feat[b], half * 180)
        m["edge"] = host_edge(NBLK, half == 0, half == 1)
        m["c1edge"] = host_c1edge(half == 0, half == 1)
        in_maps.append(m)
    return in_maps


# --------------------------------------------------------------------------
# host-side phase 2 (tiny proposal MLP)
# --------------------------------------------------------------------------

def _sine_embed(pos):
    scale = np.float32(2.0 * math.pi)
    dim_t = np.arange(128, dtype=np.float32)
    dim_t = np.power(np.float32(10000.0),
                     np.float32(2.0) * np.floor(dim_t / 2) / np.float32(128.0)
                     ).astype(np.float32)

    def emb(e):
        p = (e[..., None] * scale / dim_t).astype(np.float32)
        s = np.stack((np.sin(p[..., 0::2]), np.cos(p[..., 1::2])), axis=-1)
        return s.reshape(*p.shape[:-1], 128).astype(np.float32)

    return np.concatenate([emb(pos[..., 1]), emb(pos[..., 0])], axis=-1)


def _layernorm(x, g, b):
    m = x.mean(axis=-1, keepdims=True, dtype=np.float32)
    v = ((x - m) ** 2).mean(axis=-1, keepdims=True, dtype=np.float32)
    return ((x - m) / np.sqrt(v + np.float32(1e-5)) * g + b).astype(np.float32)


def _proposals_from_idx(feat, top_idx, w_cls, b_cls, pw1, pb1, g1, be1,
                        pw2, pb2, g2, be2):
    B = feat.shape[0]
    top_cls = top_idx // (H * W)
    top_pos = top_idx % (H * W)
    feat_flat = feat.reshape(B, C, H * W)
    qf = np.stack([feat_flat[b][:, top_pos[b]].T for b in range(B)])
    one_hot = np.eye(NC_, dtype=np.float32)[top_cls]
    qf = qf + one_hot @ w_cls.T.astype(np.float32) + b_cls
    xs = (top_pos % W).astype(np.float32) / np.float32(W)
    ys = (top_pos // W).astype(np.float32) / np.float32(H)
    pos = np.stack([xs, ys], axis=-1)
    pe = _sine_embed(pos)
    pe = np.maximum(_layernorm(pe @ pw1.T + pb1, g1, be1), 0.0)
    pe = np.maximum(_layernorm(pe @ pw2.T + pb2, g2, be2), 0.0)
    return (qf + pe).astype(np.float32)


# --------------------------------------------------------------------------
# main entry
# --------------------------------------------------------------------------

def _get_module():
    if "nc" not in _CACHE:
        _CACHE["nc"] = build_strip_module()
    return _CACHE["nc"]


def kernel(feat, w1, b1, w2, b2, w_cls, b_cls, pw1, pb1, g1, be1,
           pw2, pb2, g2, be2):
    from concourse.bass_utils import run_bass_kernel_spmd

    feat = np.ascontiguousarray(np.asarray(feat, np.float32))
    nc = _get_module()
    in_maps = _make_in_maps(feat, np.asarray(w1), np.asarray(b1),
                            np.asarray(w2), np.asarray(b2))
    res = run_bass_kernel_spmd(nc, in_maps, core_ids=list(range(8)))

    hm = np.zeros((4, NC_, H, W), np.float32)
    for c in range(8):
        b, half = divmod(c, 2)
        r0 = half * 180
        hm[b, :, r0:r0 + 180, :] = res.results[c]["hm_out"]

    top_idx = np.zeros((4, NPROP), np.int64)
    for b in range(4):
        vals_l, keys_l, tail_l = [], [], []
        for half in (0, 1):
            r = res.results[2 * b + half]
            vals, cls, y, x, valid = decode_candidates(
                r["cand_vals"], r["cand_idx"])
            y = y + half * 180
            flat = cls * (H * W) + y * W + x
            vals_l.append(vals[valid])
            keys_l.append(flat[valid])
            # 8th-largest value of each (partition, chunk) for the guard
            tail_l.append(r["cand_vals"].reshape(128, 4, 8)[:, :, 7].ravel())
        vals_a = np.concatenate(vals_l)
        keys_a = np.concatenate(keys_l)
        order = np.lexsort((keys_a, -vals_a))[:NPROP]
        need_fallback = len(order) < NPROP
        if not need_fallback:
            top_idx[b] = keys_a[order]
            # exactness guard: if any chunk's extracted 8th value still ranks
            # in the global top-200, that chunk may hide further candidates ->
            # exact top-k on the full heatmap instead (rare).
            thresh = vals_a[order][-1]
            need_fallback = bool((np.concatenate(tail_l) >= thresh).any())
        if need_fallback:
            flat_hm = hm[b].reshape(-1)
            part = np.argpartition(-flat_hm, NPROP + 256)[:NPROP + 257]
            o2 = part[np.lexsort((part, -flat_hm[part]))]
            top_idx[b] = o2[:NPROP]

    out = _proposals_from_idx(feat, top_idx,
                              np.asarray(w_cls), np.asarray(b_cls),
                              np.asarray(pw1), np.asarray(pb1),
                              np.asarray(g1), np.asarray(be1),
                              np.asarray(pw2), np.asarray(pb2),
                              np.asarray(g2), np.asarray(be2))
    return out, hm
